# revision 22
# baseline (speedup 1.0000x reference)
"""Tensor-parallel decoder layer for 8 TRN2 NeuronCores (Bass/Tile).

Sharding (Megatron-style TP, row-sharded norms):
  core c owns heads {2c, 2c+1} (QKV column shard), FFN inner slice
  [1024c, 1024c+1024), and the interleaved output rows
  { 512 q + 64 c + r : q in 0..3, r in 0..63 }  (64-row strips, one per
  512-row chunk, so the FFN2 ReduceScatter can be chunked and overlapped).

Collectives:
  AllToAll  — attention output: head-shard -> row-shard (bf16, 1 MB/core)
  AllGather — post-LN1 hidden (transposed) for FFN input (bf16, 1 MB/core)
  ReduceScatter x4 — FFN2 partial sums -> row strips (bf16, 4 MB chunks,
                     fired as FFN2 row-chunks complete, overlapping compute)

All matmuls run in bf16 with fp32 PSUM accumulation; softmax/LayerNorm
run in fp32. Softmax skips max-subtraction (scores are O(1) here) and
obtains denominators via a ones-column appended to V.
"""
import sys

if '/opt/trn_rl_repo' not in sys.path:
    sys.path.insert(0, '/opt/trn_rl_repo')

import math

import ml_dtypes
import numpy as np

D = 2048
H = 16
HD = 128
INNER = 8192
L = 2048
NC = 8
HPC = H // NC        # 2 heads per core
RPC = L // NC        # 256 rows per core
IPC = INNER // NC    # 1024 inner per core
KC = D // 128        # 16 contraction chunks of 128
MC = IPC // 128      # 8 inner chunks
NQ = 4               # row chunks (512 rows each) / RS chunks
EPS = 1e-5
ISCALE = 1.0 / math.sqrt(float(D))

BF16 = np.dtype(ml_dtypes.bfloat16)

_built = {}


def own_rows(c):
    """Global row indices owned by core c, ascending."""
    return np.concatenate([np.arange(512 * q + 64 * c, 512 * q + 64 * c + 64)
                           for q in range(NQ)])


def _layer_norm(nc, mybir, small, out_t, in_t, g_b, be_b, eps_t):
    """LayerNorm over the free axis (D): out = (in-mu)/sqrt(var+eps)*g + be."""
    fp32 = mybir.dt.float32
    mu = small.tile([128, 1], fp32, name="mu", tag="mu")
    nc.vector.reduce_sum(mu[:], in_t[:], axis=mybir.AxisListType.X)
    nc.vector.tensor_scalar_mul(mu[:], mu[:], 1.0 / D)
    cent = small.tile([128, D], fp32, name="cent", tag="cent")
    nc.vector.tensor_scalar_sub(cent[:], in_t[:], mu[:])
    sq = small.tile([128, D], fp32, name="sq", tag="sq")
    varacc = small.tile([128, 1], fp32, name="varacc", tag="varacc")
    nc.scalar.activation(sq[:], cent[:], mybir.ActivationFunctionType.Square,
                         accum_out=varacc[:])
    std = small.tile([128, 1], fp32, name="std", tag="std")
    nc.scalar.activation(std[:], varacc[:], mybir.ActivationFunctionType.Sqrt,
                         bias=eps_t[:], scale=1.0 / D)
    rstd = small.tile([128, 1], fp32, name="rstd", tag="rstd")
    nc.vector.reciprocal(rstd[:], std[:])
    # out = (cent * rstd) * g + be
    nc.vector.scalar_tensor_tensor(out_t[:], cent[:], rstd[:], g_b[:],
                                   op0=mybir.AluOpType.mult,
                                   op1=mybir.AluOpType.mult)
    nc.vector.tensor_add(out_t[:], out_t[:], be_b[:])


def _build():
    import concourse.bass as bass
    import concourse.mybir as mybir
    import concourse.tile as tile
    from concourse import bacc
    from concourse.masks import make_identity
    from contextlib import ExitStack

    fp32 = mybir.dt.float32
    bf16 = mybir.dt.bfloat16
    RG = [list(range(NC))]

    nc = bacc.Bacc("TRN2", target_bir_lowering=False, debug=False, num_devices=NC)

    # ---------------- I/O ----------------
    xT = nc.dram_tensor("xT", [D, L], bf16, kind="ExternalInput").ap()
    xr = nc.dram_tensor("xr", [RPC, D], fp32, kind="ExternalInput").ap()
    wqk_d = nc.dram_tensor("wqk", [D, 512], bf16, kind="ExternalInput").ap()
    wv = nc.dram_tensor("wv", [D, 2 * HD], bf16, kind="ExternalInput").ap()
    bqk = nc.dram_tensor("bqk", [128, 4], fp32, kind="ExternalInput").ap()
    bv = nc.dram_tensor("bv", [2 * HD], fp32, kind="ExternalInput").ap()
    wo = nc.dram_tensor("wo", [D, D], bf16, kind="ExternalInput").ap()
    bo = nc.dram_tensor("bo", [D], bf16, kind="ExternalInput").ap()
    w1_d = nc.dram_tensor("w1", [D, IPC], bf16, kind="ExternalInput").ap()
    b1r = nc.dram_tensor("b1r", [128, MC], fp32, kind="ExternalInput").ap()
    w2r = nc.dram_tensor("w2r", [IPC, D], bf16, kind="ExternalInput").ap()
    b2 = nc.dram_tensor("b2", [D], bf16, kind="ExternalInput").ap()
    g1 = nc.dram_tensor("g1", [D], bf16, kind="ExternalInput").ap()
    be1 = nc.dram_tensor("be1", [D], bf16, kind="ExternalInput").ap()
    g2 = nc.dram_tensor("g2", [D], bf16, kind="ExternalInput").ap()
    be2 = nc.dram_tensor("be2", [D], bf16, kind="ExternalInput").ap()
    out = nc.dram_tensor("out", [RPC, D], fp32, kind="ExternalOutput").ap()

    # ---------------- collective bounce buffers ----------------
    # a2a piece layout: [dst core, attn row, row-chunk q, 64 within-strip]
    a2a_in = [nc.dram_tensor(f"a2a_in{u}", [NC, 2 * HD, 2, 64], bf16).ap()
              for u in range(2)]
    a2a_out = [nc.dram_tensor(f"a2a_out{u}", [NC, 2 * HD, 2, 64], bf16).ap()
               for u in range(2)]
    ag2_in = [nc.dram_tensor(f"ag2_in{s}", [D, 128], bf16).ap() for s in range(2)]
    ag2_out = [nc.dram_tensor(f"ag2_out{s}", [NC, D, 128], bf16, addr_space="Shared").ap()
               for s in range(2)]
    rs_in = [nc.dram_tensor(f"rs_in{q}", [512, D], bf16).ap() for q in range(NQ)]
    rs_out = [nc.dram_tensor(f"rs_out{q}", [64, D], bf16).ap() for q in range(NQ)]

    def bcast_ap(vec_ap, parts=128):
        return bass.AP(
            tensor=vec_ap.tensor,
            offset=vec_ap.offset,
            ap=[[0, parts]] + list(vec_ap.ap),
        )

    with tile.TileContext(nc) as tc, ExitStack() as top:
        const = top.enter_context(tc.tile_pool(name="const", bufs=1))

        ident = const.tile([128, 128], bf16, name="ident")
        make_identity(nc, ident)

        masks = []
        for r in range(4):
            m = const.tile([128, 512], bf16, name=f"mask{r}", bufs=1)
            nc.gpsimd.memset(m, 1.0)
            # keep (1.0) iff  -i + j - 128 r >= 0   (i = partition, j = free)
            nc.gpsimd.affine_select(
                out=m, in_=m, compare_op=mybir.AluOpType.is_ge, fill=0.0,
                base=-128 * r, pattern=[[1, 512]], channel_multiplier=-1,
            )
            masks.append(m)

        eps_t = const.tile([128, 1], fp32, name="eps_t")
        nc.vector.memset(eps_t, EPS)

        bqk_sb = const.tile([128, 4], fp32, name="bqk_sb")
        nc.sync.dma_start(bqk_sb[:], bqk[:])
        b1_sb = const.tile([128, MC], fp32, name="b1_sb")
        nc.sync.dma_start(b1_sb[:], b1r[:])
        bv_sb = const.tile([128, 2 * HD], fp32, name="bv_sb")
        nc.gpsimd.dma_start(bv_sb[:], bcast_ap(bv))

        # persistent activations (live across several phases)
        acts = top.enter_context(tc.tile_pool(name="acts", bufs=1))
        h_sb = [acts.tile([128, D], fp32, name=f"h_sb{s}") for s in range(2)]
        ln_small = top.enter_context(tc.tile_pool(name="ln_small", bufs=2))

        # ====== Phases 1+2 scope: QKV and attention ======
        with ExitStack() as ph12:
            qkv_p = ph12.enter_context(tc.tile_pool(name="qkv_acts", bufs=1))
            qk_sb = [qkv_p.tile([128, L], bf16, name=f"qk_sb{m}") for m in range(4)]
            v_sb = [qkv_p.tile([128, 2 * (HD + 1)], bf16, name=f"v_sb{k}")
                    for k in range(KC)]

            # ---- Phase 1: QKV ----
            with ExitStack() as ph:
                xts_p = ph.enter_context(tc.tile_pool(name="xts", bufs=1))
                wqk_p = ph.enter_context(tc.tile_pool(name="wqk", bufs=1))
                wv_p = ph.enter_context(tc.tile_pool(name="wvp", bufs=1))
                ps_qk = ph.enter_context(tc.tile_pool(name="ps_qk", bufs=1, space="PSUM"))
                ps_v = ph.enter_context(tc.tile_pool(name="ps_v", bufs=2, space="PSUM"))

                xts, wqk, wvt = [], [], []
                for k in range(KC):
                    t = xts_p.tile([128, L], bf16, name=f"xts{k}")
                    eng = nc.sync if k % 2 == 0 else nc.scalar
                    eng.dma_start(t[:], xT[128 * k:128 * k + 128, :])
                    xts.append(t)
                    t = wqk_p.tile([128, 512], bf16, name=f"wqk{k}")
                    nc.sync.dma_start(t[:], wqk_d[128 * k:128 * k + 128, :])
                    wqk.append(t)
                    t = wv_p.tile([128, 2 * HD], bf16, name=f"wvt{k}")
                    nc.scalar.dma_start(t[:], wv[128 * k:128 * k + 128, :])
                    wvt.append(t)

                # q0,q1,k0,k1 transposed: [128 (head dim), L]
                for m in range(4):
                    pss_qk = [ps_qk.tile([128, 512], fp32, name="ps_qk",
                                         tag=f"ps_qk{lb}") for lb in range(4)]
                    for k in range(KC):
                        for lb in range(4):
                            nc.tensor.matmul(pss_qk[lb][:],
                                             wqk[k][:, 128 * m:128 * m + 128],
                                             xts[k][:, 512 * lb:512 * lb + 512],
                                             start=(k == 0), stop=(k == KC - 1))
                    for lb in range(4):
                        nc.vector.tensor_scalar_add(
                            qk_sb[m][:, 512 * lb:512 * lb + 512], pss_qk[lb][:],
                            bqk_sb[:, m:m + 1])
                # v natural [L, 2*hd] with ones columns at HD and 2*HD+1
                for lk in range(KC):
                    ps = ps_v.tile([128, 2 * HD], fp32, name="ps_v", tag="ps_v")
                    for k in range(KC):
                        nc.tensor.matmul(ps[:], xts[k][:, 128 * lk:128 * lk + 128],
                                         wvt[k][:],
                                         start=(k == 0), stop=(k == KC - 1))
                    nc.vector.tensor_add(v_sb[lk][:, 0:HD], ps[:, 0:HD], bv_sb[:, 0:HD])
                    nc.vector.tensor_add(v_sb[lk][:, HD + 1:2 * HD + 1],
                                         ps[:, HD:2 * HD], bv_sb[:, HD:2 * HD])
                    nc.vector.memset(v_sb[lk][:, HD:HD + 1], 1.0)
                    nc.vector.memset(v_sb[lk][:, 2 * HD + 1:2 * HD + 2], 1.0)

            # ---- Phase 2: attention ----
            with ExitStack() as ph:
                exp_p = ph.enter_context(tc.tile_pool(name="expp", bufs=16))
                obt_p = ph.enter_context(tc.tile_pool(name="obtp", bufs=2))
                ps_s = ph.enter_context(tc.tile_pool(name="ps_s", bufs=3, space="PSUM"))
                ps_o = ph.enter_context(tc.tile_pool(name="ps_o", bufs=2, space="PSUM"))
                ps_t = ph.enter_context(tc.tile_pool(name="ps_t", bufs=2, space="PSUM"))
                small = ph.enter_context(tc.tile_pool(name="attn_small", bufs=4))

                for b in range(4):
                    # per-block attention output, both heads: [hd, (h), lq 512]
                    obt = obt_p.tile([128, 2, 512], bf16, name="obt", tag="obt")
                    for h in range(HPC):
                        nch = 4 * b + 4
                        qT = qk_sb[h]
                        kT = qk_sb[2 + h]
                        exp_tiles = []
                        for lk in range(nch):
                            ps = ps_s.tile([128, 512], fp32, name="ps_s", tag="ps_s")
                            nc.tensor.matmul(ps[:], kT[:, 128 * lk:128 * lk + 128],
                                             qT[:, 512 * b:512 * b + 512],
                                             start=True, stop=True)
                            et = exp_p.tile([128, 512], bf16, name="expT", tag="expT")
                            nc.scalar.activation(et[:], ps[:],
                                                 mybir.ActivationFunctionType.Exp,
                                                 scale=ISCALE)
                            r = lk - 4 * b
                            if r >= 0:
                                nc.vector.tensor_mul(et[:], et[:], masks[r][:])
                            exp_tiles.append(et)
                        for sub in range(4):
                            qc = 4 * b + sub
                            po = ps_o.tile([128, HD + 1], fp32, name="ps_o", tag="ps_o")
                            for lk in range(qc + 1):
                                nc.tensor.matmul(
                                    po[:], exp_tiles[lk][:, 128 * sub:128 * sub + 128],
                                    v_sb[lk][:, (HD + 1) * h:(HD + 1) * (h + 1)],
                                    start=(lk == 0), stop=(lk == qc))
                            recip = small.tile([128, 1], fp32, name="recip", tag="recip")
                            nc.vector.reciprocal(recip[:], po[:, HD:HD + 1])
                            onat = small.tile([128, HD], bf16, name="onat", tag="onat")
                            nc.vector.tensor_scalar_mul(onat[:], po[:, 0:HD], recip[:])
                            pt = ps_t.tile([128, 128], bf16, name="ps_t", tag="ps_t")
                            nc.tensor.transpose(pt[:], onat[:], ident[:])
                            nc.scalar.copy(obt[:, h, 128 * sub:128 * sub + 128],
                                           pt[:])
                    # scatter: piece j of half u=b//2 gets cols 64 j .. 64 j + 64
                    u = b // 2
                    dst = a2a_in[u].rearrange("c (h p) q w -> c p h q w", h=2)
                    for j in range(NC):
                        eng = nc.sync if j % 2 == 0 else nc.gpsimd
                        eng.dma_start(dst[j, :, :, b % 2, :],
                                      obt.rearrange("p h (j w) -> p h j w", j=NC)[:, :, j, :])
                    if b % 2 == 1:
                        nc.gpsimd.collective_compute(
                            "AllToAll", mybir.AluOpType.bypass, replica_groups=RG,
                            ins=[a2a_in[u][:]], outs=[a2a_out[u][:]],
                        )

        # ====== Phase 3: out-proj + residual + LN1 ======
        with ExitStack() as ph:
            vec1 = ph.enter_context(tc.tile_pool(name="vec1", bufs=1))
            bo_b = vec1.tile([128, D], bf16, name="bo_b")
            nc.gpsimd.dma_start(bo_b[:], bcast_ap(bo))
            g1_b = vec1.tile([128, D], bf16, name="g1_b")
            nc.gpsimd.dma_start(g1_b[:], bcast_ap(g1))
            be1_b = vec1.tile([128, D], bf16, name="be1_b")
            nc.gpsimd.dma_start(be1_b[:], bcast_ap(be1))

            at_p = ph.enter_context(tc.tile_pool(name="atp", bufs=1))
            wo_p = ph.enter_context(tc.tile_pool(name="wop", bufs=1))
            xr_p = ph.enter_context(tc.tile_pool(name="xrp", bufs=1))
            res_p = ph.enter_context(tc.tile_pool(name="resp", bufs=1))

            # resident Wo: loads have no deps, scheduler pulls them early
            wos = []
            for k in range(KC):
                t = wo_p.tile([128, D], bf16, name=f"wo{k}")
                eng = nc.sync if k % 2 == 0 else nc.scalar
                eng.dma_start(t[:], wo[128 * k:128 * k + 128, :])
                wos.append(t)
            xrs = []
            for s in range(2):
                t = xr_p.tile([128, D], fp32, name=f"xrs{s}")
                nc.sync.dma_start(t[:], xr[128 * s:128 * s + 128, :])
                xrs.append(t)

            # half s depends only on a2a_out[s]
            a2a_vs = [a2a_out[u].rearrange("c r q w -> (c r) (q w)") for u in range(2)]
            with ExitStack() as phl:
                ps_l = phl.enter_context(tc.tile_pool(name="ps_l", bufs=1, space="PSUM"))
                ps_t2 = phl.enter_context(tc.tile_pool(name="ps_t2", bufs=2, space="PSUM"))
                for s in range(2):
                    ats = []
                    for k in range(KC):
                        t = at_p.tile([128, 128], bf16, name=f"ats{s}_{k}")
                        eng = (nc.sync, nc.scalar, nc.gpsimd)[k % 3]
                        eng.dma_start(t[:], a2a_vs[s][128 * k:128 * k + 128, :])
                        ats.append(t)
                    pss = [ps_l.tile([128, 512], fp32, name=f"ps_l{s}_{n}",
                                     tag=f"ps_l{n}") for n in range(4)]
                    for k in range(KC):
                        for n in range(4):
                            nc.tensor.matmul(pss[n][:], ats[k][:],
                                             wos[k][:, 512 * n:512 * n + 512],
                                             start=(k == 0), stop=(k == KC - 1))
                    res1 = res_p.tile([128, D], fp32, name="res1", tag="res1")
                    for n in range(4):
                        sl = slice(512 * n, 512 * n + 512)
                        nc.vector.tensor_add(res1[:, sl], pss[n][:], bo_b[:, sl])
                        nc.vector.tensor_add(res1[:, sl], res1[:, sl], xrs[s][:, sl])
                    _layer_norm(nc, mybir, ln_small, h_sb[s], res1, g1_b, be1_b, eps_t)
                    # transpose h, then AllGather this half
                    hbf = res_p.tile([128, D], bf16, name="hbf", tag="hbf")
                    nc.vector.tensor_copy(hbf[:], h_sb[s][:])
                    for kc in range(KC):
                        pt = ps_t2.tile([128, 128], bf16, name="ps_t2", tag="ps_t2")
                        nc.tensor.transpose(pt[:], hbf[:, 128 * kc:128 * kc + 128],
                                            ident[:])
                        ht = ln_small.tile([128, 128], bf16, name="ht", tag="ht")
                        if kc % 2 == 0:
                            nc.vector.tensor_copy(ht[:], pt[:])
                        else:
                            nc.scalar.copy(ht[:], pt[:])
                        heng = nc.sync if kc % 2 == 0 else nc.gpsimd
                        heng.dma_start(
                            ag2_in[s][128 * kc:128 * kc + 128, :], ht[:])
                    nc.gpsimd.collective_compute(
                        "AllGather", mybir.AluOpType.bypass, replica_groups=RG,
                        ins=[ag2_in[s][:]], outs=[ag2_out[s][:]],
                    )

        # ====== Phases 4-6: FFN1 -> FFN2 -> chunked RS -> LN2, merged ======
        tT = [acts.tile([128, L], bf16, name=f"tT{m}") for m in range(MC)]
        # columns of ag2_out[s] block c are core c's owned rows s*2+0, s*2+1:
        # global row 512 q + 64 c + w  ->  ag2_out[q//2][c, :, 64 (q%2) + w]
        ag2_v = [ag2_out[s].rearrange("c d (p w) -> d p c w", p=2) for s in range(2)]
        with ExitStack() as ph:
            vec2 = ph.enter_context(tc.tile_pool(name="vec2", bufs=1))
            b2_b = vec2.tile([128, D], bf16, name="b2_b")
            nc.gpsimd.dma_start(b2_b[:], bcast_ap(b2))
            g2_b = vec2.tile([128, D], bf16, name="g2_b")
            nc.gpsimd.dma_start(g2_b[:], bcast_ap(g2))
            be2_b = vec2.tile([128, D], bf16, name="be2_b")
            nc.gpsimd.dma_start(be2_b[:], bcast_ap(be2))

            w1_p = ph.enter_context(tc.tile_pool(name="w1p", bufs=1))
            w2_p = ph.enter_context(tc.tile_pool(name="w2p", bufs=1))
            hx_p = ph.enter_context(tc.tile_pool(name="hxp", bufs=17))
            y_p = ph.enter_context(tc.tile_pool(name="yp", bufs=2))
            fin = ph.enter_context(tc.tile_pool(name="fin", bufs=1))
            ps_f = ph.enter_context(tc.tile_pool(name="ps_f", bufs=4, space="PSUM"))
            ps_y = ph.enter_context(tc.tile_pool(name="ps_y", bufs=4, space="PSUM"))

            w1s = []
            for k in range(KC):
                t = w1_p.tile([128, IPC], bf16, name=f"w1s{k}")
                eng = nc.sync if k % 2 == 0 else nc.scalar
                eng.dma_start(t[:], w1_d[128 * k:128 * k + 128, :])
                w1s.append(t)
            w2s = []
            for k in range(MC):
                t = w2_p.tile([128, D], bf16, name=f"w2s{k}")
                eng = nc.sync if k % 2 == 0 else nc.scalar
                eng.dma_start(t[:], w2r[128 * k:128 * k + 128, :])
                w2s.append(t)

            def final_block(s):
                # rows 128 s .. 128 s + 128 = RS chunks 2s, 2s+1
                ff = fin.tile([128, D], bf16, name="ff", tag="ff")
                nc.sync.dma_start(ff[0:64, :], rs_out[2 * s][:])
                nc.sync.dma_start(ff[64:128, :], rs_out[2 * s + 1][:])
                res2 = fin.tile([128, D], fp32, name="res2", tag="res2")
                nc.vector.tensor_add(res2[:], h_sb[s][:], b2_b[:])  # hidden pre-add
                nc.vector.tensor_add(res2[:], res2[:], ff[:])
                _layer_norm(nc, mybir, ln_small, res2, res2, g2_b, be2_b, eps_t)
                nc.sync.dma_start(out[128 * s:128 * s + 128, :], res2[:])

            for lb in range(NQ):
                # FFN1 for row block lb
                hx = []
                for k in range(KC):
                    t = hx_p.tile([128, NC, 64], bf16, name="hx", tag="hx")
                    eng = (nc.sync, nc.scalar, nc.gpsimd)[k % 3]
                    eng.dma_start(t[:], ag2_v[lb // 2][128 * k:128 * k + 128, lb % 2])
                    hx.append(t)
                for m in range(MC):
                    ps = ps_f.tile([128, 512], fp32, name="ps_f", tag="ps_f")
                    for k in range(KC):
                        nc.tensor.matmul(ps[:], w1s[k][:, 128 * m:128 * m + 128],
                                         hx[k].rearrange("p a b -> p (a b)"),
                                         start=(k == 0), stop=(k == KC - 1))
                    nc.scalar.activation(tT[m][:, 512 * lb:512 * lb + 512], ps[:],
                                         mybir.ActivationFunctionType.Gelu,
                                         bias=b1_sb[:, m:m + 1])
                # FFN2 for the same rows, then fire this chunk's ReduceScatter
                for lc in range(4 * lb, 4 * lb + 4):
                    part = (lc % 4) * 128
                    pss = [ps_y.tile([128, 512], fp32, name="ps_y", tag="ps_y")
                           for _ in range(4)]
                    for k in range(MC):
                        for n in range(4):
                            nc.tensor.matmul(pss[n][:],
                                             tT[k][:, 128 * lc:128 * lc + 128],
                                             w2s[k][:, 512 * n:512 * n + 512],
                                             start=(k == 0), stop=(k == MC - 1))
                    ysb = y_p.tile([128, D], bf16, name="ysb", tag="ysb")
                    for n in range(4):
                        nc.vector.tensor_copy(ysb[:, 512 * n:512 * n + 512], pss[n][:])
                    nc.sync.dma_start(rs_in[lb][part:part + 128, :], ysb[:])
                nc.gpsimd.collective_compute(
                    "ReduceScatter", mybir.AluOpType.add, replica_groups=RG,
                    ins=[rs_in[lb][:]], outs=[rs_out[lb][:]],
                )
                if lb % 2 == 1:
                    final_block(lb // 2)

    nc.compile()
    return nc


def _get_nc():
    if "nc" not in _built:
        _built["nc"] = _build()
    return _built["nc"]


def _shard_inputs(x, Wqkv, bqkv, Wo, bo, W1, b1, W2, b2, g1, be1, g2, be2):
    f32 = np.float32
    x = np.asarray(x, f32).reshape(L, D)
    Wqkv = np.asarray(Wqkv, f32)
    bqkv = np.asarray(bqkv, f32)
    Wo_b = np.ascontiguousarray(np.asarray(Wo, f32)).astype(BF16)
    W1 = np.asarray(W1, f32)
    W2 = np.asarray(W2, f32)

    xT_b = np.ascontiguousarray(x.T).astype(BF16)
    g1 = np.ascontiguousarray(np.asarray(g1, f32)).astype(BF16)
    be1 = np.ascontiguousarray(np.asarray(be1, f32)).astype(BF16)
    g2 = np.ascontiguousarray(np.asarray(g2, f32)).astype(BF16)
    be2 = np.ascontiguousarray(np.asarray(be2, f32)).astype(BF16)
    bo = np.ascontiguousarray(np.asarray(bo, f32)).astype(BF16)
    b2 = np.ascontiguousarray(np.asarray(b2, f32)).astype(BF16)
    b1 = np.asarray(b1, f32)

    in_maps = []
    for c in range(NC):
        wq = Wqkv[:, 0 * D + 256 * c:0 * D + 256 * c + 256]
        wk = Wqkv[:, 1 * D + 256 * c:1 * D + 256 * c + 256]
        wv_ = Wqkv[:, 2 * D + 256 * c:2 * D + 256 * c + 256]
        # [D, 512]: column blocks = (q h0, q h1, k h0, k h1)
        wqk = np.ascontiguousarray(np.concatenate([wq, wk], axis=1)).astype(BF16)
        bq = bqkv[0 * D + 256 * c:0 * D + 256 * c + 256]
        bk = bqkv[1 * D + 256 * c:1 * D + 256 * c + 256]
        bv_ = bqkv[2 * D + 256 * c:2 * D + 256 * c + 256]
        bqk_m = np.stack([bq[:128], bq[128:], bk[:128], bk[128:]], axis=1)  # [128, 4]
        w1c = W1[:, IPC * c:IPC * c + IPC]
        b1c = b1[IPC * c:IPC * c + IPC]
        in_maps.append({
            "xT": xT_b,
            "xr": np.ascontiguousarray(x[own_rows(c), :]),
            "wqk": wqk,
            "wv": np.ascontiguousarray(wv_).astype(BF16),
            "bqk": np.ascontiguousarray(bqk_m),
            "bv": np.ascontiguousarray(bv_),
            "wo": Wo_b,
            "bo": bo,
            "w1": np.ascontiguousarray(w1c).astype(BF16),
            "b1r": np.ascontiguousarray(b1c.reshape(MC, 128).T),
            "w2r": np.ascontiguousarray(W2[IPC * c:IPC * c + IPC, :]).astype(BF16),
            "b2": b2, "g1": g1, "be1": be1, "g2": g2, "be2": be2,
        })
    return in_maps


def kernel(**inputs):
    from concourse.bass_utils import run_bass_kernel_spmd

    nc = _get_nc()
    in_maps = _shard_inputs(**inputs)
    res = run_bass_kernel_spmd(nc, in_maps, list(range(NC)))
    full = np.zeros((L, D), np.float32)
    for c in range(NC):
        full[own_rows(c), :] = res.results[c]["out"]
    return full.reshape(1, L, D)
